# revision 1
# baseline (speedup 1.0000x reference)
"""Trainium2 Bass kernel for nn_PositionEncoding (embedding lookup + sincos
position encoding + mask select).

Strategy (pure data parallel across 8 cores, 65536 tokens/core):
  - out[t, 2i]   = sin(2^i * pi * v_t)
    out[t, 2i+1] = cos(2^i * pi * v_t)     (i = 0..31)
    overwritten by E_class[class_ids[t]] where is_class[t] == 1.
  - The fp32 reference angle factorizes exactly: fl32(v * 2^i*pi) = 2^i * w,
    w = fl32(pi * v).  In "turns" space tau_i = 2^(i-1) * (w/pi).  The host
    precomputes per-token group residues r_g = (2^(g*8-1) * w/pi) mod 1 in
    float64 (g = 0..3), so the device only does an EXACT power-of-two multiply
    t = F * r_g (F = 2^(i - 8g) <= 128), a magic-number round
    s = (t + 2^23) - 2^23, wrap u = t - s in [-0.5, 0.5], and the ACT `Sin`
    spline:  sin = Sin(2pi*u), cos = Sin(pi/2 - 2pi*|u|)  (args within the
    spline's +-4 domain).
  - Class rows come from `dma_gather` (SWDGE indirect DMA, 256B/row) and are
    merged with `copy_predicated`.

Per-core layout: 8 tiles x 8192 tokens; tile token (p, j) = p*64 + j
(p = partition, j = 0..63) so stores are 16KB-contiguous per partition.
The gather's position i lands at partition i%128, block i//128, and reads
index slot (i%16, i//16) of the [16, 512] wrapped idx layout -- the host
permutes class_ids accordingly.
"""
import os
os.environ.setdefault("JAX_PLATFORMS", "axon")
import math
import numpy as np

import concourse.bacc as bacc
import concourse.bass as bass
import concourse.mybir as mybir
from concourse.library_config import mlp

B, S = 64, 8192
L = 32                 # encode levels
E = 64                 # 2*L
CLASS_NUM = 4096
NCORES = 8
TPC = B * S // NCORES  # tokens per core = 65536
NTILE = 8
TT = TPC // NTILE      # tokens per tile = 8192
NB = 64                # tokens per partition per tile
NG = 4                 # level groups
NSG = 8                # gather splits per tile (packet/ring limits)
CH = TT // NSG         # indices per dma_gather
GL = 8                 # levels per group

PI32 = np.float32(math.pi)
MAGIC = float(np.float32(2.0 ** 23))

_CACHED_NC = None


def _build_nc():
    nc = bacc.Bacc("TRN2", debug=False)
    f32, i32, i16 = mybir.dt.float32, mybir.dt.int32, mybir.dt.int16
    Alu = mybir.AluOpType

    tbl = nc.dram_tensor("tbl", [CLASS_NUM + 1, E], f32, kind="ExternalInput")
    resid = nc.dram_tensor("resid", [NTILE * 128, NG * NB], f32, kind="ExternalInput")
    idx = nc.dram_tensor("idx", [NTILE * 128, TT // 16], i16, kind="ExternalInput")
    msk = nc.dram_tensor("msk", [NTILE * 128, NB], f32, kind="ExternalInput")
    fcst = nc.dram_tensor("fcst", [128, L], f32, kind="ExternalInput")
    out = nc.dram_tensor("out", [NTILE * 128, NB * E], f32, kind="ExternalOutput")

    HW = NB * L            # half-width free size (2048): one slot per (j, level)
    FW = NB * E            # full width (4096)

    from contextlib import ExitStack
    with ExitStack() as _es:
        def sb(name, shape, dt):
            return _es.enter_context(nc.sbuf_tensor(name, shape, dt))

        def sem(name):
            return _es.enter_context(nc.semaphore(name))

        f_sb = sb("f_sb", [128, L], f32)
        pi2_sb = sb("pi2_sb", [128, 1], f32)
        r0 = sb("r0", [128, NG * NB], f32); r1 = sb("r1", [128, NG * NB], f32)
        i0 = sb("i0", [128, TT // 16], i16); i1 = sb("i1", [128, TT // 16], i16)
        m0 = sb("m0", [128, NB], f32); m1 = sb("m1", [128, NB], f32)
        t0 = sb("t0", [128, HW], f32); t1 = sb("t1", [128, HW], f32)
        s0 = sb("s0", [128, HW], f32); s1 = sb("s1", [128, HW], f32)
        e0 = sb("e0", [128, FW], f32); e1 = sb("e1", [128, FW], f32)
        g0 = sb("g0", [128, FW], f32); g1 = sb("g1", [128, FW], f32)
        lr = [sem("lr0"), sem("lr1")]   # resid loads, per buffer: +16 per use
        lm = [sem("lm0"), sem("lm1")]   # msk loads
        li = [sem("li0"), sem("li1")]   # idx loads
        gd = [sem("gd0"), sem("gd1")]   # gathers
        st = [sem("st0"), sem("st1")]   # stores
        vt = sem("vt")    # DVE t-mults: +4 per tile
        vu = sem("vu")    # DVE u ready: +1 per tile
        ad = sem("ad")    # ACT passes: +3 per tile
        vp = sem("vp")    # predicated merge done: +1 per tile
        cs = sem("cs")    # consts ready

        rbuf = [r0, r1]
        ibuf = [i0, i1]
        mbuf = [m0, m1]
        tbuf = [t0, t1]
        sbuf_ = [s0, s1]
        ebuf = [e0, e1]
        gbuf = [g0, g1]

        with nc.Block() as block:

            @block.sync
            def _(sync):
                sync.dma_start(f_sb[:], fcst[:]).then_inc(cs, 16)

                def loads(k):
                    b = k % 2
                    if k >= 2:
                        # resid consumed by t-mults of tile k-2; msk reuse is
                        # covered by the preceding store(k-2) wait (vp >= k-1).
                        sync.wait_ge(vt, 4 * (k - 1))
                    sync.dma_start(
                        rbuf[b][:], resid[k * 128:(k + 1) * 128, :]
                    ).then_inc(lr[b], 16)
                    sync.dma_start(
                        mbuf[b][:], msk[k * 128:(k + 1) * 128, :]
                    ).then_inc(lm[b], 16)

                loads(0)
                loads(1)
                for k in range(NTILE):
                    b = k % 2
                    # store of tile k (enc buffer free once DMA read completes)
                    sync.wait_ge(vp, k + 1)
                    sync.dma_start(
                        out[k * 128:(k + 1) * 128, :], ebuf[b][:]
                    ).then_inc(st[b], 16)
                    if k + 2 < NTILE:
                        loads(k + 2)
                sync.wait_ge(st[0], 16 * (NTILE // 2))
                sync.wait_ge(st[1], 16 * (NTILE // 2))

            @block.gpsimd
            def _(gpsimd):
                gpsimd.load_library(mlp)
                gpsimd.memset(pi2_sb[:], float(PI32 / 2)).then_inc(cs, 1)
                for k in range(NTILE):
                    b = k % 2
                    if k >= 2:
                        # idx buffer released at gather(k-2) DMA completion
                        gpsimd.wait_ge(gd[b], 16 * NSG * (k // 2))
                    gpsimd.dma_start(
                        ibuf[b][:], idx[k * 128:(k + 1) * 128, :]
                    ).then_inc(li[b], 16)
                    if k >= 2:
                        # g buffer consumed by merge of tile k-2
                        gpsimd.wait_ge(vp, k - 1)
                    gpsimd.wait_ge(li[b], 16 * (k // 2 + 1))
                    for c in range(NSG):
                        gpsimd.dma_gather(
                            bass.AP(gbuf[b], c * (CH // 128) * E,
                                    [[FW, 128], [E, CH // 128], [1, E]]),
                            tbl[:],
                            bass.AP(ibuf[b], c * (CH // 16),
                                    [[TT // 16, 128], [1, CH // 16]]),
                            CH, CH, E, single_packet=False,
                        ).then_inc(gd[b], 16)

            @block.vector
            def _(vector):
                vector.wait_ge(cs, 17)
                for k in range(NTILE):
                    b = k % 2
                    vector.wait_ge(lr[b], 16 * (k // 2 + 1))  # resid loaded
                    if k >= 2:
                        vector.wait_ge(ad, 3 * k - 3)     # t/s read by ACT k-2
                    t, s, e, g, r, m = tbuf[b], sbuf_[b], ebuf[b], gbuf[b], rbuf[b], mbuf[b]
                    # t[p, j*32 + g*8 + l] = F[g*8+l] * r[p, g*64 + j]
                    for gi in range(NG):
                        vector.tensor_tensor(
                            bass.AP(t, gi * GL, [[HW, 128], [L, NB], [1, GL]]),
                            bass.AP(f_sb, gi * GL, [[L, 128], [0, NB], [1, GL]]),
                            bass.AP(r, gi * NB, [[NG * NB, 128], [1, NB], [0, GL]]),
                            Alu.mult,
                        ).then_inc(vt, 1)
                    vector.drain()
                    # s = round_even(t) via (t + 2^23) - 2^23
                    vector.tensor_scalar(
                        s[:], t[:], MAGIC, MAGIC, Alu.add, Alu.subtract)
                    vector.drain()
                    # u = t - s  (wrapped turns in [-0.5, 0.5]), in place over t
                    vector.tensor_tensor(
                        t[:], t[:], s[:], Alu.subtract).then_inc(vu, 1)
                    # merge: e = e * (1-m) + g   (g is zero where !m via
                    # the zero row appended to the table)
                    vector.wait_ge(ad, 3 * (k + 1))
                    vector.wait_ge(gd[b], 16 * NSG * (k // 2 + 1))
                    vector.wait_ge(lm[b], 16 * (k // 2 + 1))  # msk loaded
                    vector.tensor_tensor(
                        e[:], e[:],
                        bass.AP(m, 0, [[NB, 128], [1, NB], [0, E]]),
                        Alu.mult,
                    )
                    vector.drain()
                    vector.tensor_tensor(
                        e[:], e[:], g[:], Alu.add,
                    ).then_inc(vp, 1)

            @block.scalar
            def _(scalar):
                scalar.wait_ge(cs, 17)
                for k in range(NTILE):
                    b = k % 2
                    t, s, e = tbuf[b], sbuf_[b], ebuf[b]
                    scalar.wait_ge(vu, k + 1)
                    if k >= 2:
                        scalar.wait_ge(st[b], 16 * (k // 2))  # enc buffer stored
                    # even cols: sin = Sin(2pi * u)
                    scalar.activation(
                        bass.AP(e, 0, [[FW, 128], [E, NB], [2, L]]),
                        t[:].rearrange("p (j l) -> p j l", l=L),
                        mybir.ActivationFunctionType.Sin,
                        bias=0.0, scale=float(2.0 * PI32),
                    ).then_inc(ad, 1)
                    # s = |u|  (round values in s no longer needed)
                    scalar.activation(
                        s[:], t[:], mybir.ActivationFunctionType.Abs,
                        bias=0.0, scale=1.0,
                    ).then_inc(ad, 1)
                    # sem (not drain): enforce Abs writeback before the read
                    scalar.wait_ge(ad, 3 * k + 2)
                    # odd cols: cos = Sin(-2pi * |u| + pi/2)
                    scalar.activation(
                        bass.AP(e, 1, [[FW, 128], [E, NB], [2, L]]),
                        s[:].rearrange("p (j l) -> p j l", l=L),
                        mybir.ActivationFunctionType.Sin,
                        bias=pi2_sb[:, 0:1], scale=float(-2.0 * PI32),
                    ).then_inc(ad, 1)

    nc.compile()
    return nc


def _host_prep(values, E_class, class_ids, is_class):
    """Split across cores and build device-layout input arrays."""
    v = np.ascontiguousarray(values, dtype=np.float32).reshape(-1)
    ids = np.ascontiguousarray(class_ids, dtype=np.int32).reshape(-1)
    m = np.ascontiguousarray(is_class, dtype=np.int32).reshape(-1)

    w = (v * PI32).astype(np.float32)
    q = w.astype(np.float64) / np.float64(math.pi)
    # group residues, float64 -> f32
    resid_full = np.empty((NG, v.size), np.float32)
    for g in range(NG):
        resid_full[g] = np.mod(q * (2.0 ** (g * GL - 1)), 1.0).astype(np.float32)

    # gather position permutation within a tile:
    # position i -> token (i%128)*64 + i//128 ; idx slot (r=i%16, c=i//16)
    i_arr = np.arange(TT, dtype=np.int64)
    tok_of_pos = (i_arr % 128) * NB + i_arr // 128   # [8192]

    tbl_pad = np.concatenate(
        [np.asarray(E_class, dtype=np.float32),
         np.zeros((1, E), np.float32)], axis=0)
    fcst = np.broadcast_to(
        (np.float32(2.0) ** (np.arange(L, dtype=np.float32) % GL)), (128, L)
    ).copy()

    in_maps = []
    for c in range(NCORES):
        sl = slice(c * TPC, (c + 1) * TPC)
        rc = resid_full[:, sl]                        # [4, 65536]
        idc = ids[sl]
        mc = m[sl]

        # resid device layout [tile*128 + p, g*64 + j]
        # token (tile, p, j) = tile*8192 + p*64 + j
        r_t = rc.reshape(NG, NTILE, 128, NB)          # [g, tile, p, j]
        r_dev = np.ascontiguousarray(
            r_t.transpose(1, 2, 0, 3).reshape(NTILE * 128, NG * NB))

        m_dev = np.ascontiguousarray(
            (1.0 - mc.astype(np.float32)).reshape(NTILE * 128, NB))

        # idx device layout: per tile [16, 512] wrapped, tiled to 128 rows
        idm = np.where(mc != 0, idc, CLASS_NUM)      # zero row when !is_class
        idt = idm.reshape(NTILE, TT)
        idx_dev = np.empty((NTILE, 128, TT // 16), np.int16)
        for ktile in range(NTILE):
            vals = idt[ktile][tok_of_pos]             # value for position i
            wrap = vals.reshape(TT // 16, 16).T       # [16, 512]: slot (r,c)=pos c*16+r
            idx_dev[ktile] = np.tile(wrap, (8, 1)).astype(np.int16)
        idx_dev = idx_dev.reshape(NTILE * 128, TT // 16)

        in_maps.append({
            "tbl": tbl_pad,
            "resid": r_dev,
            "idx": idx_dev,
            "msk": m_dev,
            "fcst": fcst,
        })
    return in_maps


def kernel(values, E_class, class_ids, is_class):
    global _CACHED_NC
    if _CACHED_NC is None:
        _CACHED_NC = _build_nc()
    nc = _CACHED_NC

    in_maps = _host_prep(values, E_class, class_ids, is_class)

    from concourse.bass_utils import run_bass_kernel_spmd
    res = run_bass_kernel_spmd(nc, in_maps, core_ids=list(range(NCORES)))

    outs = []
    for c in range(NCORES):
        o = res.results[c]["out"]                     # [1024, 4096]
        # [tile*128+p, j*64+d] -> token (tile*8192 + p*64 + j), d
        outs.append(o.reshape(TPC, E))
    full = np.concatenate(outs, axis=0)               # [524288, 64]
    return full.reshape(B, S, E)



# revision 11
# speedup vs baseline: 1.1174x; 1.1174x over previous
"""Trainium2 Bass kernel for nn_PositionEncoding (embedding lookup + sincos
position encoding + mask select).

Strategy (v2): the host re-deals tokens across the 8 cores x 8 tiles.
Class tokens are grouped into QUADS sharing one class id (tokens sorted by
id; ~64 tokens/class so ~98% pack), and every tile receives exactly Q_u
quads placed at gather descriptors 0..Q_u-1.  Descriptor i writes SBUF slot
(partition i%128, j = 4*(i//128)..+3) -- 512B of a pre-quadrupled fp16 table
row, i.e. 4 token embeddings per descriptor at full DMA-bus efficiency.

  - DVE computes wrapped turns u = F*r - round(F*r) with ONE fused custom
    DVE op per level group (mult + magic-round + subtract in a single
    4-stage pass), and the cos argument w = 0.25 - |u| with a second 7-stage
    fused op.  The fp32 angle factorization is exact: fl32(v * 2^i*pi)
    = 2^i * fl32(pi*v); the host precomputes group residues
    r_g = (2^(8g-1) * fl32(pi*v)/pi) mod 1 in float64.
  - ACT writes sin = Sin(2pi*u) to even columns and cos = Sin(2pi*w) to odd
    columns of the fp16 output tile -- only for j-blocks >= j0 = 4*(Q_u//128)
    (blocks below j0 are entirely class tokens and would be overwritten).
  - dma_gather (SWDGE) writes the class embeddings DIRECTLY into the output
    tile, overwriting sincos where the class region partially covers
    j0..j0+3.  No mask, no merge op.
  - The store writes fp16 (half traffic); the host scatters rows back to the
    original token order and upcasts, and patches the ~2% leftover class
    tokens (unpaired quad remainders) straight from E_class.

Rel-err budget: fp16 output quantization ~3e-4 << 2e-2 gate.
"""
import os
os.environ.setdefault("JAX_PLATFORMS", "axon")
import math
import numpy as np

import concourse.bacc as bacc
import concourse.bass as bass
import concourse.mybir as mybir
from concourse.library_config import mlp

B, S = 64, 8192
L = 32                 # encode levels
E = 64                 # 2*L
CLASS_NUM = 4096
NCORES = 8
TPC = B * S // NCORES  # tokens per core = 65536
NTILE = 8
TT = TPC // NTILE      # tokens per tile = 8192
NB = 64                # tokens per partition per tile
NG = 4                 # level groups
GL = 8                 # levels per group
QE = 4                 # tokens per gather descriptor (quad)
CHMAX = 1024           # max descriptors per dma_gather (SWDGE ring carveout)

PI32 = np.float32(math.pi)
MAGIC = float(np.float32(2.0 ** 23))

# ---------------------------------------------------------------- custom DVE
# op1: u = m - round_even(m),  m = f * r   (magic-number round; all fp32)
# op2: w = 0.25 - |u|                      (cos argument in turns)
from concourse.dve_spec import Spec, Src0, Src1, C0, C1, Zero, maxx
from concourse.dve_ops import DveOp
import concourse.dve_ops as _dve_ops_mod

_m = Src0 * Src1
_u = _m - ((_m + C0) - C0)


def _ref_u(in0, in1, s0, s1, imm2):
    m = (np.asarray(in0, np.float32) * np.asarray(in1, np.float32)).astype(np.float32)
    a = (m + np.float32(s0)).astype(np.float32)
    s = (a - np.float32(s0)).astype(np.float32)
    return (m - s).astype(np.float32)


def _ref_w(in0, in1, s0, s1, imm2):
    u = _ref_u(in0, in1, s0, s1, imm2)
    return (np.float32(s1) - np.abs(u)).astype(np.float32)


MULFRAC_ANT = DveOp(
    "MULFRAC_ANT",
    Spec(body=_u, reference=_ref_u),
    subdim=False,
    uops_sha={"v3": "bf0a82bb185299f4", "v4": "135b89f34ee1a84c"},
)
MULFRACCOS_ANT = DveOp(
    "MULFRACCOS_ANT",
    Spec(body=C1 - maxx(_u, Zero - _u), reference=_ref_w),
    subdim=False,
    uops_sha={"v3": "58802921ee5ecc62", "v4": "ceec7d44b5e9e35c"},
)

for _op in (MULFRAC_ANT, MULFRACCOS_ANT):
    if not any(o.name == _op.name for o in _dve_ops_mod.OPS):
        _dve_ops_mod.OPS.append(_op)
        _dve_ops_mod.CUSTOM_DVE_SPECS[_op.name] = _op.spec
        _dve_ops_mod._SUB_OPCODE_FOR_NAME[_op.name] = (
            _dve_ops_mod._CUSTOM_DVE_ROW_BASE + len(_dve_ops_mod.OPS) - 1)

_CACHED = {}   # (Q_u,) -> compiled nc


def _build_nc(Q_u):
    j0 = QE * (Q_u // 128)            # j-blocks fully covered by quads
    JW = NB - j0                      # j-blocks that need sincos
    HW = JW * L                       # free width of t/s buffers
    KCOLS = Q_u // 16                 # idx columns per tile
    splits = []
    pos = 0
    while pos < Q_u:
        ln = min(CHMAX, Q_u - pos)
        splits.append((pos, ln))
        pos += ln
    nsp = len(splits)

    nc = bacc.Bacc("TRN2", debug=False)
    f32, f16, i16 = mybir.dt.float32, mybir.dt.float16, mybir.dt.int16

    tbl = nc.dram_tensor("tbl", [CLASS_NUM, QE * E], f16, kind="ExternalInput")
    resid = nc.dram_tensor("resid", [NTILE * 128, NG * JW], f32, kind="ExternalInput")
    idx = nc.dram_tensor("idx", [NTILE * 128, KCOLS], i16, kind="ExternalInput")
    fcst = nc.dram_tensor("fcst", [128, L], f32, kind="ExternalInput")
    out = nc.dram_tensor("out", [NTILE * 128, NB * E], f16, kind="ExternalOutput")

    from contextlib import ExitStack
    with ExitStack() as _es:
        def sb(name, shape, dt):
            return _es.enter_context(nc.sbuf_tensor(name, shape, dt))

        def sem(name):
            return _es.enter_context(nc.semaphore(name))

        f_sb = sb("f_sb", [128, L], f32)
        r0 = sb("r0", [128, NG * JW], f32); r1 = sb("r1", [128, NG * JW], f32)
        i0 = sb("i0", [128, KCOLS], i16); i1 = sb("i1", [128, KCOLS], i16)
        t0 = sb("t0", [128, HW], f32); t1 = sb("t1", [128, HW], f32)
        s0 = sb("s0", [128, HW], f32); s1 = sb("s1", [128, HW], f32)
        e0 = sb("e0", [128, NB * E], f16); e1 = sb("e1", [128, NB * E], f16)
        lr = [sem("lr0"), sem("lr1")]   # resid loads: +16 each
        li = [sem("li0"), sem("li1")]   # idx loads
        gd = [sem("gd0"), sem("gd1")]   # gather DMA completions
        st = [sem("st0"), sem("st1")]   # stores
        va = sem("va")    # u (t buffer) ready: +1 per tile
        vb = sem("vb")    # w (s buffer) ready: +1 per tile
        ad = sem("ad")    # ACT passes: +2 per tile (sin, cos in order)
        cs = sem("cs")    # f_sb loaded

        rbuf = [r0, r1]
        ibuf = [i0, i1]
        tbuf = [t0, t1]
        sbuf_ = [s0, s1]
        ebuf = [e0, e1]

        with nc.Block() as block:

            @block.sync
            def _(sync):
                sync.dma_start(f_sb[:], fcst[:]).then_inc(cs, 16)

                def loads(k):
                    b = k % 2
                    if k >= 2:
                        # rbuf[b] consumed once tile k-2's DVE ops all done
                        sync.wait_ge(vb, k - 1)
                    sync.dma_start(
                        rbuf[b][:], resid[k * 128:(k + 1) * 128, :]
                    ).then_inc(lr[b], 16)

                loads(0)
                loads(1)
                for k in range(NTILE):
                    b = k % 2
                    sync.wait_ge(gd[b], 16 * nsp * (k // 2 + 1))
                    sync.dma_start(
                        out[k * 128:(k + 1) * 128, :], ebuf[b][:]
                    ).then_inc(st[b], 16)
                    if k + 2 < NTILE:
                        loads(k + 2)
                sync.wait_ge(st[0], 16 * (NTILE // 2))
                sync.wait_ge(st[1], 16 * (NTILE // 2))

            @block.gpsimd
            def _(gpsimd):
                gpsimd.load_library(mlp)
                for k in range(NTILE):
                    b = k % 2
                    if k >= 2:
                        # ibuf[b] free once tile k-2's gathers completed
                        gpsimd.wait_ge(gd[b], 16 * nsp * (k // 2))
                    gpsimd.dma_start(
                        ibuf[b][:], idx[k * 128:(k + 1) * 128, :]
                    ).then_inc(li[b], 16)
                    gpsimd.wait_ge(li[b], 16 * (k // 2 + 1))
                    # class rows overwrite sincos -> wait both ACT passes
                    gpsimd.wait_ge(ad, 2 * k + 2)
                    for (pos, ln) in splits:
                        gpsimd.dma_gather(
                            bass.AP(ebuf[b], (pos // 128) * QE * E,
                                    [[NB * E, 128],
                                     [QE * E, (ln + 127) // 128], [1, QE * E]]),
                            bass.AP(tbl, 0, [[QE * E, CLASS_NUM], [1, QE * E]]),
                            bass.AP(ibuf[b], pos // 16,
                                    [[KCOLS, 128], [1, (ln + 15) // 16]]),
                            ln, ln, QE * E, elem_step=QE * E,
                            single_packet=False,
                        ).then_inc(gd[b], 16)

            @block.vector
            def _(vector):
                vector.wait_ge(cs, 16)
                for k in range(NTILE):
                    b = k % 2
                    t, s, r = tbuf[b], sbuf_[b], rbuf[b]
                    vector.wait_ge(lr[b], 16 * (k // 2 + 1))
                    if k >= 2:
                        vector.wait_ge(ad, 2 * k - 3)   # sin(k-2) read t
                    for gi in range(NG):
                        h = vector._custom_dve(
                            MULFRAC_ANT,
                            out=bass.AP(t, gi * GL, [[HW, 128], [L, JW], [1, GL]]),
                            in0=bass.AP(f_sb, gi * GL, [[L, 128], [0, JW], [1, GL]]),
                            in1=bass.AP(r, gi * JW, [[NG * JW, 128], [1, JW], [0, GL]]),
                            s0=MAGIC,
                        )
                        if gi == NG - 1:
                            h.then_inc(va, 1)
                    if k >= 2:
                        vector.wait_ge(ad, 2 * k - 2)   # cos(k-2) read s
                    for gi in range(NG):
                        h = vector._custom_dve(
                            MULFRACCOS_ANT,
                            out=bass.AP(s, gi * GL, [[HW, 128], [L, JW], [1, GL]]),
                            in0=bass.AP(f_sb, gi * GL, [[L, 128], [0, JW], [1, GL]]),
                            in1=bass.AP(r, gi * JW, [[NG * JW, 128], [1, JW], [0, GL]]),
                            s0=MAGIC, s1=0.25,
                        )
                        if gi == NG - 1:
                            h.then_inc(vb, 1)

            @block.scalar
            def _(scalar):
                for k in range(NTILE):
                    b = k % 2
                    t, s, e = tbuf[b], sbuf_[b], ebuf[b]
                    scalar.wait_ge(va, k + 1)
                    if k >= 2:
                        scalar.wait_ge(st[b], 16 * (k // 2))  # ebuf stored
                    scalar.activation(
                        bass.AP(e, j0 * E, [[NB * E, 128], [E, JW], [2, L]]),
                        bass.AP(t, 0, [[HW, 128], [L, JW], [1, L]]),
                        mybir.ActivationFunctionType.Sin,
                        bias=0.0, scale=float(2.0 * PI32),
                    ).then_inc(ad, 1)
                    scalar.wait_ge(vb, k + 1)
                    scalar.activation(
                        bass.AP(e, j0 * E + 1, [[NB * E, 128], [E, JW], [2, L]]),
                        bass.AP(s, 0, [[HW, 128], [L, JW], [1, L]]),
                        mybir.ActivationFunctionType.Sin,
                        bias=0.0, scale=float(2.0 * PI32),
                    ).then_inc(ad, 1)

    nc.compile()
    return nc


def _host_prep(values, E_class, class_ids, is_class):
    """Quad-group class tokens, re-deal across 64 tiles, build device arrays.

    Returns (in_maps, token_for_slot[64, 8192], leftover_tokens, Q_u, tbl16).
    """
    v = np.ascontiguousarray(values, dtype=np.float32).reshape(-1)
    ids = np.ascontiguousarray(class_ids, dtype=np.int32).reshape(-1)
    mk = np.ascontiguousarray(is_class, dtype=np.int32).reshape(-1) != 0

    NTILES_G = NCORES * NTILE                       # 64 global tiles

    # --- group class tokens into same-id quads -----------------------------
    cls_tok = np.flatnonzero(mk)
    cids = ids[cls_tok]
    so = np.argsort(cids, kind="stable")
    T = cls_tok[so]
    C = cids[so]
    # position within each equal-id run
    change = np.empty(C.size, bool)
    change[0] = True
    change[1:] = C[1:] != C[:-1]
    rstart_of = np.maximum.accumulate(np.where(change, np.arange(C.size), 0))
    pos_in_run = np.arange(C.size) - rstart_of
    runlen = np.diff(np.r_[np.flatnonzero(change), C.size])
    runlen_of = np.repeat(runlen, runlen)
    keep = pos_in_run < (runlen_of // QE) * QE
    Tq = T[keep]                                    # quad tokens, 4 per id-run
    NQ = Tq.size // QE
    Q_u = (NQ // NTILES_G) // 16 * 16               # quads per tile
    assert Q_u > 0
    K_u = QE * Q_u                                  # class tokens per tile
    j0 = QE * (Q_u // 128)
    JW = NB - j0

    quads = Tq[: NQ * QE].reshape(NQ, QE)
    used_q = quads[: NTILES_G * Q_u].reshape(NTILES_G, Q_u, QE)
    qids = ids[used_q[:, :, 0]].astype(np.int16)    # [64, Q_u]

    # leftover class tokens: unpaired remainders + unused quads (host-patched)
    leftover = np.concatenate([T[~keep], quads[NTILES_G * Q_u:].reshape(-1)])

    # --- slot assignment ---------------------------------------------------
    i_arr = np.arange(Q_u, dtype=np.int64)
    slots_q = ((i_arr % 128) * NB + QE * (i_arr // 128))[:, None] + \
        np.arange(QE, dtype=np.int64)[None, :]      # [Q_u, 4]
    slot_mask = np.ones(TT, bool)
    slot_mask[slots_q.reshape(-1)] = False
    rest_slots = np.flatnonzero(slot_mask)          # all have j >= j0

    nonclass = np.flatnonzero(~mk)
    rest_pool = np.concatenate([leftover, nonclass])
    R_u = TT - K_u
    assert rest_pool.size == NTILES_G * R_u
    rest_chunk = rest_pool.reshape(NTILES_G, R_u)

    tfs = np.empty((NTILES_G, TT), np.int64)        # token-for-slot
    tfs[:, slots_q.reshape(-1)] = used_q.reshape(NTILES_G, K_u)
    tfs[:, rest_slots] = rest_chunk

    # --- residues only for slots with j >= j0 ------------------------------
    need = tfs.reshape(NTILES_G, 128, NB)[:, :, j0:]            # [64,128,JW]
    w = (v * PI32).astype(np.float32)
    q = w.astype(np.float64) / np.float64(math.pi)
    qn = q[need]
    resid = np.empty((NTILES_G, 128, NG, JW), np.float32)
    for g in range(NG):
        resid[:, :, g, :] = np.mod(qn * (2.0 ** (g * GL - 1)), 1.0)
    resid = resid.reshape(NTILES_G, 128, NG * JW)

    # idx: descriptor i at (row i%16, col i//16), replicated to 128 partitions
    # (real SWDGE reads the 16-partition wrap from every 16-partition stripe)
    idx_dev = np.ascontiguousarray(np.tile(
        qids.reshape(NTILES_G, Q_u // 16, 16).transpose(0, 2, 1), (1, 8, 1)))

    tbl16 = np.asarray(E_class, dtype=np.float16)               # [4096, 64]
    tbl4 = np.ascontiguousarray(
        np.broadcast_to(tbl16[:, None, :], (CLASS_NUM, QE, E))
    ).reshape(CLASS_NUM, QE * E)
    fcst = np.broadcast_to(
        (np.float32(2.0) ** (np.arange(L, dtype=np.float32) % GL)), (128, L)
    ).copy()

    in_maps = []
    for c in range(NCORES):
        tl = slice(c * NTILE, (c + 1) * NTILE)
        in_maps.append({
            "tbl": tbl4,
            "resid": np.ascontiguousarray(resid[tl].reshape(NTILE * 128, NG * JW)),
            "idx": np.ascontiguousarray(idx_dev[tl].reshape(NTILE * 128, Q_u // 16)),
            "fcst": fcst,
        })

    return in_maps, tfs, leftover, Q_u, tbl16


def kernel(values, E_class, class_ids, is_class):
    in_maps, tfs, leftover, Q_u, tbl16 = _host_prep(
        values, E_class, class_ids, is_class)

    if Q_u not in _CACHED:
        _CACHED[Q_u] = _build_nc(Q_u)
    nc = _CACHED[Q_u]

    from concourse.bass_utils import run_bass_kernel_spmd
    res = run_bass_kernel_spmd(nc, in_maps, core_ids=list(range(NCORES)))

    ids = np.ascontiguousarray(class_ids, dtype=np.int32).reshape(-1)
    dev = np.stack([res.results[c]["out"] for c in range(NCORES)])  # [8,1024,4096]
    dev = dev.reshape(NCORES * NTILE, TT, E)

    full16 = np.empty((B * S, E), np.float16)
    full16[tfs.reshape(-1)] = dev.reshape(-1, E)
    if leftover.size:
        full16[leftover] = tbl16[ids[leftover]]
    return full16.astype(np.float32).reshape(B, S, E)


# revision 62
# speedup vs baseline: 1.8040x; 1.6145x over previous
"""Trainium2 Bass kernel for nn_PositionEncoding (embedding lookup + sincos
position encoding + mask select).

Strategy (v2): the host re-deals tokens across the 8 cores x 8 tiles.
Class tokens are grouped into QUADS sharing one class id (tokens sorted by
id; ~64 tokens/class so ~98% pack), and every tile receives exactly Q_u
quads placed at gather descriptors 0..Q_u-1.  Descriptor i writes SBUF slot
(partition i%128, j = 4*(i//128)..+3) -- 512B of a pre-quadrupled fp16 table
row, i.e. 4 token embeddings per descriptor at full DMA-bus efficiency.

  - DVE computes wrapped turns u = F*r - round(F*r) with ONE fused custom
    DVE op per level group (mult + magic-round + subtract in a single
    4-stage pass), and the cos argument w = 0.25 - |u| with a second 7-stage
    fused op.  The fp32 angle factorization is exact: fl32(v * 2^i*pi)
    = 2^i * fl32(pi*v); the host precomputes group residues
    r_g = (2^(8g-1) * fl32(pi*v)/pi) mod 1 in float64.
  - ACT writes sin = Sin(2pi*u) to even columns and cos = Sin(2pi*w) to odd
    columns of the fp16 output tile -- only for j-blocks >= j0 = 4*(Q_u//128)
    (blocks below j0 are entirely class tokens and would be overwritten).
  - dma_gather (SWDGE) writes the class embeddings DIRECTLY into the output
    tile, overwriting sincos where the class region partially covers
    j0..j0+3.  No mask, no merge op.
  - The store writes fp16 (half traffic); the host scatters rows back to the
    original token order and upcasts, and patches the ~2% leftover class
    tokens (unpaired quad remainders) straight from E_class.

Rel-err budget: fp16 output quantization ~3e-4 << 2e-2 gate.
"""
import os
os.environ.setdefault("JAX_PLATFORMS", "axon")
import math
import numpy as np

import concourse.bacc as bacc
import concourse.bass as bass
import concourse.mybir as mybir
from concourse.library_config import mlp

B, S = 64, 8192
L = 32                 # encode levels
E = 64                 # 2*L
CLASS_NUM = 4096
NCORES = 8
TPC = B * S // NCORES  # tokens per core = 65536
NTILE = 8
TT = TPC // NTILE      # tokens per tile = 8192
NB = 64                # tokens per partition per tile
NG = 4                 # level groups
GL = 8                 # levels per group
QE = 4                 # tokens per gather descriptor (quad)
CHMAX = 1024           # max descriptors per dma_gather (SWDGE ring carveout)
NBUF = 4               # pipeline depth (tiles in flight)

PI32 = np.float32(math.pi)
MAGIC = float(np.float32(2.0 ** 23))

# ---------------------------------------------------------------- custom DVE
# un = round_even(m) - m = -u,  m = f * r   (magic-number round; all fp32).
# The negation is free here and lets the shared Sin activation use scale
# -2pi for both halves: Sin(-2pi*un) = sin(2pi*u) and, with the stock-op
# follow-up w = |un| - 0.25, Sin(-2pi*w) = cos(2pi*u).
from concourse.dve_spec import Spec, Src0, Src1, C0, C1, Zero, maxx
from concourse.dve_ops import DveOp
import concourse.dve_ops as _dve_ops_mod

_m = Src0 * Src1
_un = ((_m + C0) - C0) - _m


def _ref_un(in0, in1, s0, s1, imm2):
    m = (np.asarray(in0, np.float32) * np.asarray(in1, np.float32)).astype(np.float32)
    a = (m + np.float32(s0)).astype(np.float32)
    s = (a - np.float32(s0)).astype(np.float32)
    return (s - m).astype(np.float32)


def _ref_wn(in0, in1, s0, s1, imm2):
    un = _ref_un(in0, in1, s0, s1, imm2)
    return (np.abs(un) - np.float32(s1)).astype(np.float32)


MULFRACN_ANT = DveOp(
    "MULFRACN_ANT",
    Spec(body=_un, reference=_ref_un),
    subdim=False,
    uops_sha={"v3": "45b2546aa893c0b3", "v4": "e9640e257af8fa7d"},
)
MULFRACCOSN_ANT = DveOp(
    "MULFRACCOSN_ANT",
    Spec(body=maxx(_un, Zero - _un) - C1, reference=_ref_wn),
    subdim=False,
    uops_sha={"v3": "8c6b8a0a0537ce82", "v4": "f2dc81f150e31cd6"},
)

for _op in (MULFRACN_ANT, MULFRACCOSN_ANT):
    if not any(o.name == _op.name for o in _dve_ops_mod.OPS):
        _dve_ops_mod.OPS.append(_op)
        _dve_ops_mod.CUSTOM_DVE_SPECS[_op.name] = _op.spec
        _dve_ops_mod._SUB_OPCODE_FOR_NAME[_op.name] = (
            _dve_ops_mod._CUSTOM_DVE_ROW_BASE + len(_dve_ops_mod.OPS) - 1)

_CACHED = {}   # (Q_u,) -> compiled nc


def _build_nc(Q_u):
    assert Q_u % 128 == 0             # no partial group: gather writes j < j0
    j0 = QE * (Q_u // 128)            # j-blocks fully covered by quads
    JW = NB - j0                      # j-blocks that need sincos
    HW = JW * L                       # elems per h-half of the ts buffer
    KCOLS = Q_u // 16                 # idx columns per tile
    splits = []
    pos = 0
    while pos < Q_u:
        ln = min(CHMAX, Q_u - pos)
        splits.append((pos, ln))
        pos += ln
    nsp = len(splits)
    # stores spread across the three DMA-capable engines; ACT takes the last
    # tile (it is idle right after producing the final activation)
    SP_STORES = [0, 1, 2, 4, 6]
    POOL_STORES = [3, 5]
    ACT_STORES = [NTILE - 1]

    nc = bacc.Bacc("TRN2", debug=False)
    f32, f16, i16 = mybir.dt.float32, mybir.dt.float16, mybir.dt.int16

    tbl = nc.dram_tensor("tbl", [CLASS_NUM, QE * E], f16, kind="ExternalInput")
    # resid/idx: all tiles side by side along the free dim (merged loads)
    resid = nc.dram_tensor("resid", [128, NTILE * JW * NG], f32,
                           kind="ExternalInput")
    idx = nc.dram_tensor("idx", [128, NTILE * KCOLS], i16, kind="ExternalInput")
    fcst = nc.dram_tensor("fcst", [128, L], f32, kind="ExternalInput")
    out = nc.dram_tensor("out", [NTILE * 128, NB * E], f16, kind="ExternalOutput")

    from contextlib import ExitStack
    with ExitStack() as _es:
        def sb(name, shape, dt):
            return _es.enter_context(nc.sbuf_tensor(name, shape, dt))

        def sem(name):
            return _es.enter_context(nc.semaphore(name))

        f_sb = sb("f_sb", [128, L], f32)
        rbig = sb("rbig", [128, NTILE * JW * NG], f32)
        ibig = sb("ibig", [128, NTILE * KCOLS], i16)
        tsbuf = [sb(f"ts{i}", [128, 2 * HW], f32) for i in range(NTILE)]
        ebuf = [sb(f"e{i}", [128, NB * E], f16) for i in range(NTILE)]
        lr0 = sem("lr0")  # resid tile 0 loaded
        lrA = sem("lrA")  # resid tiles 1..3 loaded
        lrB = sem("lrB")  # resid tiles 4..7 loaded
        li = sem("li")    # idx loaded (single DMA)
        gd = [sem(f"gd{i}") for i in range(NTILE)]   # gathers per tile
        st = sem("st")    # SP stores
        stp = sem("stp")  # Pool stores (software-DGE needs its own sem)
        va = sem("va")    # u half of ts ready: +1 per tile
        vb = sem("vb")    # w half of ts ready: +1 per tile
        ad = sem("ad")    # ACT pass done: +1 per tile
        cs = sem("cs")    # f_sb loaded

        # ts layout: ts[p, ((j*NG+g)*GL + l)*2 + h]; h=0 holds -u, h=1 holds
        # |u|-0.25.  Sin(-2pi*x) of the whole buffer lands contiguously at
        # output column j0*64 + ts-index (sin even cols, cos odd cols).
        def store(eng, k, s):
            eng.wait_ge(ad, 2 * k + 2)
            eng.wait_ge(gd[k], 16 * nsp)
            eng.dma_start(
                out[k * 128:(k + 1) * 128, :], ebuf[k][:]
            ).then_inc(s, 16)

        with nc.Block() as block:

            @block.sync
            def _(sync):
                sync.dma_start(f_sb[:], fcst[:]).then_inc(cs, 16)
                for k in SP_STORES:
                    store(sync, k, st)
                sync.wait_ge(st, 16 * (len(SP_STORES) + len(ACT_STORES)))
                sync.wait_ge(stp, 16 * len(POOL_STORES))

            @block.scalar
            def _(scalar):
                for k in range(NTILE):
                    ts, e = tsbuf[k], ebuf[k]
                    scalar.wait_ge(va, k + 1)
                    scalar.activation(
                        bass.AP(e, j0 * E, [[NB * E, 128], [2, HW]]),
                        bass.AP(ts, 0, [[2 * HW, 128], [2, HW]]),
                        mybir.ActivationFunctionType.Sin,
                        bias=0.0, scale=float(-2.0 * PI32),
                    ).then_inc(ad, 1)
                    scalar.wait_ge(vb, k + 1)
                    scalar.activation(
                        bass.AP(e, j0 * E + 1, [[NB * E, 128], [2, HW]]),
                        bass.AP(ts, 1, [[2 * HW, 128], [2, HW]]),
                        mybir.ActivationFunctionType.Sin,
                        bias=0.0, scale=float(-2.0 * PI32),
                    ).then_inc(ad, 1)
                for k in ACT_STORES:
                    store(scalar, k, st)

            @block.gpsimd
            def _(gpsimd):
                gpsimd.load_library(mlp)
                RW = JW * NG
                half = NTILE // 2
                gpsimd.dma_start(
                    rbig[:, :RW], resid[:, :RW]).then_inc(lr0, 16)
                gpsimd.dma_start(ibig[:], idx[:]).then_inc(li, 16)
                gpsimd.dma_start(
                    rbig[:, RW:half * RW], resid[:, RW:half * RW]
                ).then_inc(lrA, 16)
                gpsimd.dma_start(
                    rbig[:, half * RW:], resid[:, half * RW:]
                ).then_inc(lrB, 16)
                gpsimd.wait_ge(li, 16)
                for k in range(NTILE):
                    # gathers write j < j0 only -- no ACT dependency
                    for (pos, ln) in splits:
                        gpsimd.dma_gather(
                            bass.AP(ebuf[k], (pos // 128) * QE * E,
                                    [[NB * E, 128],
                                     [QE * E, (ln + 127) // 128], [1, QE * E]]),
                            bass.AP(tbl, 0, [[QE * E, CLASS_NUM], [1, QE * E]]),
                            bass.AP(ibig, k * KCOLS + pos // 16,
                                    [[NTILE * KCOLS, 128], [1, (ln + 15) // 16]]),
                            ln, ln, QE * E, elem_step=QE * E,
                            single_packet=False,
                        ).then_inc(gd[k], 16)
                for k in POOL_STORES:
                    store(gpsimd, k, stp)

            @block.vector
            def _(vector):
                vector.wait_ge(cs, 16)
                Alu = mybir.AluOpType
                for k in range(NTILE):
                    ts = tsbuf[k]
                    if k == 0:
                        vector.wait_ge(lr0, 16)
                    elif k == 1:
                        vector.wait_ge(lrA, 16)
                    elif k == NTILE // 2:
                        vector.wait_ge(lrB, 16)
                    vector._custom_dve(
                        MULFRACN_ANT,
                        out=bass.AP(ts, 0, [[2 * HW, 128], [2 * GL, JW * NG], [2, GL]]),
                        in0=bass.AP(f_sb, 0, [[L, 128], [0, JW * NG], [1, GL]]),
                        in1=bass.AP(rbig, k * JW * NG,
                                    [[NTILE * JW * NG, 128], [1, JW * NG], [0, GL]]),
                        s0=MAGIC,
                    ).then_inc(va, 1)
                    # w = |un| - 0.25, recomputed from (f, r) -- independent
                    # of op_un, so no intra-engine RAW wait is needed
                    vector._custom_dve(
                        MULFRACCOSN_ANT,
                        out=bass.AP(ts, 1, [[2 * HW, 128], [2 * GL, JW * NG], [2, GL]]),
                        in0=bass.AP(f_sb, 0, [[L, 128], [0, JW * NG], [1, GL]]),
                        in1=bass.AP(rbig, k * JW * NG,
                                    [[NTILE * JW * NG, 128], [1, JW * NG], [0, GL]]),
                        s0=MAGIC, s1=0.25,
                    ).then_inc(vb, 1)

    nc.compile()
    return nc


def _host_prep(values, E_class, class_ids, is_class):
    """Quad-group class tokens, re-deal across 64 tiles, build device arrays.

    Returns (in_maps, token_for_slot[64, 8192], leftover_tokens, Q_u, tbl16).
    """
    v = np.ascontiguousarray(values, dtype=np.float32).reshape(-1)
    ids = np.ascontiguousarray(class_ids, dtype=np.int32).reshape(-1)
    mk = np.ascontiguousarray(is_class, dtype=np.int32).reshape(-1) != 0

    NTILES_G = NCORES * NTILE                       # 64 global tiles

    # --- group class tokens into same-id quads -----------------------------
    cls_tok = np.flatnonzero(mk)
    cids = ids[cls_tok]
    so = np.argsort(cids, kind="stable")
    T = cls_tok[so]
    C = cids[so]
    # position within each equal-id run
    change = np.empty(C.size, bool)
    change[0] = True
    change[1:] = C[1:] != C[:-1]
    rstart_of = np.maximum.accumulate(np.where(change, np.arange(C.size), 0))
    pos_in_run = np.arange(C.size) - rstart_of
    runlen = np.diff(np.r_[np.flatnonzero(change), C.size])
    runlen_of = np.repeat(runlen, runlen)
    keep = pos_in_run < (runlen_of // QE) * QE
    Tq = T[keep]                                    # quad tokens, 4 per id-run
    NQ = Tq.size // QE
    Q_u = (NQ // NTILES_G) // 128 * 128             # quads per tile
    assert Q_u > 0
    K_u = QE * Q_u                                  # class tokens per tile
    j0 = QE * (Q_u // 128)
    JW = NB - j0

    quads = Tq[: NQ * QE].reshape(NQ, QE)
    used_q = quads[: NTILES_G * Q_u].reshape(NTILES_G, Q_u, QE)
    qids = ids[used_q[:, :, 0]].astype(np.int16)    # [64, Q_u]

    # leftover class tokens: unpaired remainders + unused quads (host-patched)
    leftover = np.concatenate([T[~keep], quads[NTILES_G * Q_u:].reshape(-1)])

    # --- slot assignment ---------------------------------------------------
    i_arr = np.arange(Q_u, dtype=np.int64)
    slots_q = ((i_arr % 128) * NB + QE * (i_arr // 128))[:, None] + \
        np.arange(QE, dtype=np.int64)[None, :]      # [Q_u, 4]
    slot_mask = np.ones(TT, bool)
    slot_mask[slots_q.reshape(-1)] = False
    rest_slots = np.flatnonzero(slot_mask)          # all have j >= j0

    nonclass = np.flatnonzero(~mk)
    rest_pool = np.concatenate([leftover, nonclass])
    R_u = TT - K_u
    assert rest_pool.size == NTILES_G * R_u
    rest_chunk = rest_pool.reshape(NTILES_G, R_u)

    tfs = np.empty((NTILES_G, TT), np.int64)        # token-for-slot
    tfs[:, slots_q.reshape(-1)] = used_q.reshape(NTILES_G, K_u)
    tfs[:, rest_slots] = rest_chunk

    # --- residues only for slots with j >= j0 ------------------------------
    need = tfs.reshape(NTILES_G, 128, NB)[:, :, j0:]            # [64,128,JW]
    w = (v * PI32).astype(np.float32)
    q = w.astype(np.float64) / np.float64(math.pi)
    qn = q[need]
    # j-major layout: resid[tile, p, j', g] (matches the fused ts nesting)
    resid = np.empty((NTILES_G, 128, JW, NG), np.float32)
    for g in range(NG):
        resid[:, :, :, g] = np.mod(qn * (2.0 ** (g * GL - 1)), 1.0)
    resid = resid.reshape(NTILES_G, 128, JW * NG)

    # idx: descriptor i at (row i%16, col i//16), replicated to 128 partitions
    # (real SWDGE reads the 16-partition wrap from every 16-partition stripe)
    idx_dev = np.ascontiguousarray(np.tile(
        qids.reshape(NTILES_G, Q_u // 16, 16).transpose(0, 2, 1), (1, 8, 1)))

    tbl16 = np.asarray(E_class, dtype=np.float16)               # [4096, 64]
    tbl4 = np.ascontiguousarray(
        np.broadcast_to(tbl16[:, None, :], (CLASS_NUM, QE, E))
    ).reshape(CLASS_NUM, QE * E)
    fcst = np.broadcast_to(
        (np.float32(2.0) ** (np.arange(L, dtype=np.float32) % GL)), (128, L)
    ).copy()

    in_maps = []
    for c in range(NCORES):
        tl = slice(c * NTILE, (c + 1) * NTILE)
        # tile-major along the free dim: [128, tile * width + col]
        r_core = np.ascontiguousarray(
            resid[tl].transpose(1, 0, 2).reshape(128, NTILE * JW * NG))
        i_core = np.ascontiguousarray(
            idx_dev[tl].transpose(1, 0, 2).reshape(128, NTILE * (Q_u // 16)))
        in_maps.append({
            "tbl": tbl4,
            "resid": r_core,
            "idx": i_core,
            "fcst": fcst,
        })

    return in_maps, tfs, leftover, Q_u, tbl16


def kernel(values, E_class, class_ids, is_class):
    in_maps, tfs, leftover, Q_u, tbl16 = _host_prep(
        values, E_class, class_ids, is_class)

    if Q_u not in _CACHED:
        _CACHED[Q_u] = _build_nc(Q_u)
    nc = _CACHED[Q_u]

    from concourse.bass_utils import run_bass_kernel_spmd
    res = run_bass_kernel_spmd(nc, in_maps, core_ids=list(range(NCORES)))

    ids = np.ascontiguousarray(class_ids, dtype=np.int32).reshape(-1)
    dev = np.stack([res.results[c]["out"] for c in range(NCORES)])  # [8,1024,4096]
    dev = dev.reshape(NCORES * NTILE, TT, E)

    full16 = np.empty((B * S, E), np.float16)
    full16[tfs.reshape(-1)] = dev.reshape(-1, E)
    if leftover.size:
        full16[leftover] = tbl16[ids[leftover]]
    return full16.astype(np.float32).reshape(B, S, E)
